# revision 42
# baseline (speedup 1.0000x reference)
"""Trainium2 Bass kernel for EnhancedTripletLoss (hard-mining triplet loss).

Strategy (8 NeuronCores, SPMD, no collectives):
  * Rows (anchors) are sharded BY CLASS: core c handles all anchors of class c
    (8 classes == 8 cores), padded to a uniform 128-aligned slab height Mc.
  * Columns (candidates) are permuted into 8 uniform 1024-wide class blocks
    (small classes padded with duplicate columns -- duplicates never change a
    min/max); the excess columns of classes larger than 1024 live in one
    shared OVERFLOW region whose per-class minima become extra bmins columns.
  * Per core, fp8(e4m3) DoubleRow matmuls compute
        g[a, j] = <fp8(-2 e_a), fp8(e_j)> + sqj        (sqj = ||e_j||^2)
    into PSUM: one K=256 DR matmul (both K-chunks packed) plus one K=4 DR
    matmul carrying sqj as four fp8 terms of sq/4 (stationary constant 4.0).
    The core's OWN class columns are sign-negated on the v side, so a single
    free-dim min per block yields both hard-positive and hard-negative stats:
        own block:    min(-g) = -(max over positives of (d2 - sqa))
        other blocks: min( g) =  (min over that block's negatives of (d2-sqa))
    ||e_a||^2 rides per-row in fp32 and is folded in after the reduce.
    fp8 1-term quantization gives loss rel-err ~5e-4 (validated vs fp32 ref).
  * Reduce pipeline is split across engines so the DVE is no longer the
    bottleneck: blocks 0-5 are evacuated fp32->fp16 by the Scalar (Act)
    engine, then min-reduced by a batched DVE tensor_tensor-min halving tree
    (fp16 SBUF hits the DVE 2x packed mode); blocks 6-7 and the overflow
    region are tensor_reduce'd directly from PSUM on the DVE.  fp16 rounding
    commutes with min (monotone), so it only perturbs the final value by
    ~2^-11 relative.
  * All per-anchor-tile epilogue math (block masks, sqrt, margin, masking) is
    deferred to ONE batched pass after the loop.
  * Each core writes per-partition partial sums [128, 2] (loss numerator,
    valid count); the host does the final tiny sum + divide.
"""

import numpy as np
import ml_dtypes

P = 128          # SBUF partitions
D = 256          # embedding dim (fixed by the problem)
NCLS = 8         # number of classes == number of cores
NCORES = 8
MARGIN = 0.3
BIGM = 1.0e30    # block-exclusion additive mask (applied to bmins stats only)
W = 1024         # uniform class-block width: 2 fp32 PSUM banks
NEVAC = 6        # blocks evacuated via Act engine + DVE fp16 tree
SQTERMS = 3      # bf16 terms for the ||e_j||^2 channel

BF16 = ml_dtypes.bfloat16


def _layout(counts):
    """Overflow classes/widths from class counts (main blocks are uniform W)."""
    ov = [(c, int(n) - W) for c, n in enumerate(counts) if n > W]
    ovw = sum(w for _, w in ov)
    assert ovw <= 512, f"overflow region too wide: {ovw}"
    return tuple(ov)


def _build_program(Mc, ov):
    import concourse.tile as tile
    from concourse import bacc, mybir

    f32 = mybir.dt.float32
    f16 = mybir.dt.float16
    bf16 = mybir.dt.bfloat16
    AX = mybir.AxisListType.X
    OP = mybir.AluOpType
    COPY = mybir.ActivationFunctionType.Copy

    Mt = Mc // P
    ovw = sum(w for _, w in ov)
    NB = NCLS + len(ov)
    N = NCLS * W + ovw

    nc = bacc.Bacc("TRN2", target_bir_lowering=False, debug=False)

    NW = NCLS * W
    v0d = nc.dram_tensor("v0", [P, NW], bf16, kind="ExternalInput")
    v1d = nc.dram_tensor("v1", [P, NW], bf16, kind="ExternalInput")
    v2d = nc.dram_tensor("v2", [SQTERMS, NW], bf16, kind="ExternalInput")
    if ovw:
        v0od = nc.dram_tensor("v0ov", [P, ovw], bf16, kind="ExternalInput")
        v1od = nc.dram_tensor("v1ov", [P, ovw], bf16, kind="ExternalInput")
        v2od = nc.dram_tensor("v2ov", [SQTERMS, ovw], bf16,
                              kind="ExternalInput")
    u0d = nc.dram_tensor("u0", [P, Mc], bf16, kind="ExternalInput")
    u1d = nc.dram_tensor("u1", [P, Mc], bf16, kind="ExternalInput")
    out = nc.dram_tensor("out", [P, Mt * NB], f32, kind="ExternalOutput")

    with tile.TileContext(nc) as tc:
        with (
            tc.tile_pool(name="resident", bufs=1) as res,
            tc.tile_pool(name="psum", bufs=3, space="PSUM") as pp,
            tc.tile_pool(name="povf", bufs=2, space="PSUM") as po,
            tc.tile_pool(name="evac", bufs=2) as ev,
            tc.tile_pool(name="tree", bufs=2) as tr,
        ):
            # ---- PE warmup ------------------------------------------------
            # dummy matmuls during the DMA fill so the PE's HAM clock-gate
            # reaches 8/8 (2.4 GHz) before the real stream starts.
            wsrc = res.tile([P, 512], bf16, tag="wsrc")
            nc.vector.memset(wsrc[:], 0.0)
            wp = pp.tile([P, W], f32, tag="pblk", name="warm")
            for _ in range(5):
                nc.tensor.matmul(wp[:, 0:512], wsrc[:, 0:P], wsrc[:, :],
                                 start=True, stop=True)

            # ---- resident loads -------------------------------------------
            dma_engs = [nc.sync, nc.gpsimd, nc.scalar]
            _dma_rr = [0]

            def dma(out_ap, in_ap):
                dma_engs[_dma_rr[0] % len(dma_engs)].dma_start(
                    out=out_ap, in_=in_ap)
                _dma_rr[0] += 1

            u0t = res.tile([P, Mc], bf16, tag="u0")
            dma(u0t[:], u0d[:, :])
            u1t = res.tile([P, Mc], bf16, tag="u1")
            dma(u1t[:], u1d[:, :])
            u2t = res.tile([32 + SQTERMS, Mc], bf16, tag="u2")
            nc.vector.memset(u2t[:], 1.0)
            # V moving operands: consolidated tiles, DMA'd per block in
            # consumption order so block b lands just before the PE needs it
            v0t = res.tile([P, NW], bf16, tag="v0", name="v0")
            v1t = res.tile([P, NW], bf16, tag="v1", name="v1")
            v2t = res.tile([32 + SQTERMS, NW], bf16, tag="v2", name="v2")
            for rp in (0, 32):
                dma(v2t[rp:rp + SQTERMS, :], v2d[:, :])
            for b in range(NCLS):
                cs = slice(b * W, (b + 1) * W)
                dma(v0t[:, cs], v0d[:, cs])
                dma(v1t[:, cs], v1d[:, cs])
            if ovw:
                ov0 = res.tile([P, ovw], bf16, tag="ov0")
                dma(ov0[:], v0od[:, :])
                ov1 = res.tile([P, ovw], bf16, tag="ov1")
                dma(ov1[:], v1od[:, :])
                ov2 = res.tile([SQTERMS, ovw], bf16, tag="ov2")
                dma(ov2[:], v2od[:, :])

            bmall = res.tile([P, Mt, NB], f32, tag="bmall")

            # ---- main loop ------------------------------------------------
            for mt in range(Mt):
                ms = slice(mt * P, (mt + 1) * P)
                evt = ev.tile([P, NEVAC, W], f16, tag="ev")
                for b in range(NCLS):
                    ptile = pp.tile([P, W], f32, tag="pblk", name="pblk")
                    for ti, (ut, vt) in enumerate(((u0t, v0t), (u1t, v1t))):
                        for s in range(2):
                            cl = slice(s * 512, (s + 1) * 512)
                            cg = slice(b * W + s * 512,
                                       b * W + (s + 1) * 512)
                            nc.tensor.matmul(
                                ptile[:, cl], ut[:, ms], vt[:, cg],
                                start=(ti == 0), stop=False,
                            )
                    for s in range(2):
                        cl = slice(s * 512, (s + 1) * 512)
                        cg = slice(b * W + s * 512, b * W + (s + 1) * 512)
                        rp = 32 * (s % 2)
                        nc.tensor.matmul(
                            ptile[:, cl],
                            u2t[rp:rp + SQTERMS, ms],
                            v2t[rp:rp + SQTERMS, cg],
                            start=False, stop=True,
                            tile_position=(rp, 0),
                        )
                    if b >= NCLS - NEVAC:
                        nc.scalar.activation(
                            evt[:, b - (NCLS - NEVAC), :], ptile[:, :], COPY)
                    else:
                        nc.vector.tensor_reduce(
                            bmall[:, mt, b: b + 1], ptile[:, :],
                            axis=AX, op=OP.min,
                        )

                if ovw:
                    otile = po.tile([P, ovw], f32, tag="ovf", name="ovf")
                    nc.tensor.matmul(
                        otile[:, :], u0t[:, ms], ov0[:, :],
                        start=True, stop=False,
                    )
                    nc.tensor.matmul(
                        otile[:, :], u1t[:, ms], ov1[:, :],
                        start=False, stop=False,
                    )
                    nc.tensor.matmul(
                        otile[:, :], u2t[0:SQTERMS, ms], ov2[:, :],
                        start=False, stop=True,
                    )
                    oo = 0
                    for k, (cls, w) in enumerate(ov):
                        nc.vector.tensor_reduce(
                            bmall[:, mt, NCLS + k: NCLS + k + 1],
                            otile[:, oo:oo + w], axis=AX, op=OP.min,
                        )
                        oo += w

                # fp16 min tree over the evacuated blocks (DVE 2x packed)
                t1 = tr.tile([P, NEVAC, 512], f16, tag="t1")
                nc.vector.tensor_tensor(
                    t1[:, :, :], evt[:, :, 0:512], evt[:, :, 512:1024],
                    op=OP.min)
                t2 = tr.tile([P, NEVAC, 256], f16, tag="t2")
                nc.vector.tensor_tensor(
                    t2[:, :, :], t1[:, :, 0:256], t1[:, :, 256:512],
                    op=OP.min)
                t3 = tr.tile([P, NEVAC, 128], f16, tag="t3")
                nc.vector.tensor_tensor(
                    t3[:, :, :], t2[:, :, 0:128], t2[:, :, 128:256],
                    op=OP.min)
                nc.vector.tensor_reduce(
                    bmall[:, mt, NCLS - NEVAC:NCLS], t3[:, :, :],
                    axis=AX, op=OP.min)

            # epilogue (mask/sqrt/margin/sum) runs host-side on bmall
            nc.sync.dma_start(out=out[:, :], in_=bmall[:, :, :])

    nc.compile()
    return nc


def _bf16_terms(x, nterms):
    """Decompose fp32 array into bf16 terms summing to ~x."""
    terms = []
    r = x.astype(np.float32)
    for _ in range(nterms):
        h = r.astype(BF16)
        terms.append(h)
        r = r - h.astype(np.float32)
    return terms


def _prepare_inputs(emb, lab):
    """Host-side shard/layout prep.  Returns (in_maps, meta)."""
    B = emb.shape[0]
    assert emb.shape[1] == D
    counts = np.bincount(lab, minlength=NCLS).astype(int)
    assert counts.sum() == B

    order = np.argsort(lab, kind="stable")
    cstart = np.concatenate([[0], np.cumsum(counts)]).astype(int)

    ov = _layout(counts)
    ovw = sum(w for _, w in ov)
    NB = NCLS + len(ov)
    Mc = int(((max(1, counts.max()) + P - 1) // P) * P)
    Mt = Mc // P
    N = NCLS * W + ovw

    sq = np.einsum("ij,ij->i", emb, emb, dtype=np.float32)  # ||e||^2, fp32

    # column index: uniform W-wide main blocks (dup-padded), then overflow
    colidx = np.empty(N, dtype=np.int64)
    own_ranges = {c: [] for c in range(NCLS)}
    for c in range(NCLS):
        idx = order[cstart[c]:cstart[c + 1]][:W]
        if len(idx) == 0:
            idx = order[0:1]  # arbitrary real point; class is invalid anyway
        reps = int(np.ceil(W / len(idx)))
        blk = np.tile(idx, reps)[:W]
        colidx[c * W:(c + 1) * W] = blk
        own_ranges[c].append((c * W, W))
    off = NCLS * W
    for cls, w in ov:
        idx = order[cstart[cls] + W:cstart[cls + 1]]
        assert len(idx) == w
        colidx[off:off + w] = idx
        own_ranges[cls].append((off, w))
        off += w

    # bf16 operands (shared across cores before sign application)
    Vg = np.ascontiguousarray(emb[colidx].T).astype(BF16)    # [256, N]
    sq_terms = _bf16_terms(sq, SQTERMS)
    sqf_t = np.stack([t[colidx] for t in sq_terms])          # [SQTERMS, N]

    u_full = (-2.0 * emb).astype(BF16)  # [B, 256]

    bm_cls = list(range(NCLS)) + [cls for cls, _ in ov]

    in_maps = []
    host = []
    for c in range(NCLS):
        aidx = order[cstart[c]:cstart[c + 1]]
        if len(aidx) == 0:
            aidx = order[0:1]
        npad = Mc - len(aidx)
        pad = np.full(npad, aidx[0], dtype=np.int64)
        aidx_p = np.concatenate([aidx, pad])

        real = np.zeros(Mc, dtype=np.float32)
        real[: min(len(aidx), Mc)] = 1.0
        cls_valid = 1.0 if (2 <= counts[c] <= B - 1) else 0.0
        valid = (real * cls_valid).reshape(Mt, P).T.copy()  # [128, Mt]

        sqa_t = sq[aidx_p].reshape(Mt, P).T.copy()          # [128, Mt]

        s = np.ones(N, dtype=np.float32)
        for o, w in own_ranges[c]:
            s[o:o + w] = -1.0
        sb = s.astype(BF16)  # +-1 exact

        posbig = np.zeros((NB,), dtype=np.float32)
        negbig = np.zeros((NB,), dtype=np.float32)
        for j, bc in enumerate(bm_cls):
            if bc == c:
                negbig[j] = BIGM
            else:
                posbig[j] = BIGM

        uT = u_full[aidx_p].T  # [256, Mc] bf16
        vv0 = (Vg[0:128] * sb).astype(BF16)
        vv1 = (Vg[128:256] * sb).astype(BF16)
        vv2 = (sqf_t * sb).astype(BF16)
        im = {
            "u0": np.ascontiguousarray(uT[0:128]),
            "u1": np.ascontiguousarray(uT[128:256]),
            "v0": np.ascontiguousarray(vv0[:, :NCLS * W]),
            "v1": np.ascontiguousarray(vv1[:, :NCLS * W]),
            "v2": np.ascontiguousarray(vv2[:, :NCLS * W]),
        }
        if ovw:
            im["v0ov"] = np.ascontiguousarray(vv0[:, NCLS * W:])
            im["v1ov"] = np.ascontiguousarray(vv1[:, NCLS * W:])
            im["v2ov"] = np.ascontiguousarray(vv2[:, NCLS * W:])
        in_maps.append(im)
        host.append(dict(sqa=sqa_t, valid=valid, posbig=posbig,
                         negbig=negbig))

    meta = dict(Mc=Mc, ov=ov, Mt=Mt, N=N, NB=NB, host=host)
    return in_maps, meta


_PROGRAM_CACHE = {}


def _get_program(Mc, ov):
    key = (Mc, ov)
    if key not in _PROGRAM_CACHE:
        _PROGRAM_CACHE[key] = _build_program(Mc, ov)
    return _PROGRAM_CACHE[key]


def _combine(results, meta):
    """Host-side epilogue: block-mask the bmins, fold sqa, sqrt, margin."""
    Mt, NB = meta["Mt"], meta["NB"]
    num = 0.0
    den = 0.0
    for r, h in zip(results, meta["host"]):
        bm = np.asarray(r["out"], dtype=np.float32).reshape(P, Mt, NB)
        mown = (bm + h["posbig"][None, None, :]).min(axis=2)  # [128, Mt]
        mneg = (bm + h["negbig"][None, None, :]).min(axis=2)
        pd = np.sqrt(np.maximum(-mown + h["sqa"], 0.0))
        nd = np.sqrt(np.maximum(mneg + h["sqa"], 0.0))
        per = np.maximum(pd - nd + MARGIN, 0.0)
        num += float((per * h["valid"]).sum())
        den += float(h["valid"].sum())
    return np.float32(num / max(den, 1.0))


def _setup_trace_hook():
    """Register the axon NTFF profile hook if the image lacks antenv.axon_hooks."""
    import sys
    import types
    try:
        from antenv.axon_hooks import get_axon_ntff_profile_hook  # noqa: F401
        return
    except ImportError:
        pass
    import antenv
    from trn_agent_boot.trn_boot import _ntff_profile_via_ctypes

    mod = types.ModuleType("antenv.axon_hooks")
    state = {"h": None}
    mod.set_axon_ntff_profile_hook = lambda h: state.__setitem__("h", h)
    mod.get_axon_ntff_profile_hook = lambda: state["h"]
    sys.modules["antenv.axon_hooks"] = mod
    antenv.axon_hooks = mod
    mod.set_axon_ntff_profile_hook(
        _ntff_profile_via_ctypes("/opt/axon/libaxon_pjrt.so")
    )


def kernel(embeddings, labels, _trace=False):
    emb = np.ascontiguousarray(np.asarray(embeddings, dtype=np.float32))
    lab = np.asarray(labels).astype(np.int64).ravel()

    in_maps, meta = _prepare_inputs(emb, lab)
    nc = _get_program(meta["Mc"], meta["ov"])

    from concourse.bass_utils import run_bass_kernel_spmd

    if _trace:
        _setup_trace_hook()
        import concourse.bass_utils as _bu
        _bu.upload_artifacts = lambda tmpdir: tmpdir  # skip remote upload

    res = run_bass_kernel_spmd(
        nc, in_maps, core_ids=list(range(NCORES)), trace=bool(_trace),
    )
    loss = _combine(res.results, meta)
    if _trace:
        return loss, res
    return loss
